# revision 1
# baseline (speedup 1.0000x reference)
"""Trainium2 Bass kernel for ConvReshapeBefore (im2col patch extraction).

Full problem: x (32, 64, 64, 64) f32 NHWC, kernel 3x3 stride 1 valid ->
out (62*62*32, 3, 3, 64) f32 where out[(r*62+c)*32 + b] = x[b, r:r+3, c:c+3, :].

Sharding: data-parallel over batch, 4 batches per core across 8 cores.

Per-core pipeline (measured: small-descriptor DMA paths cap at 26-136 GB/s,
big-descriptor SWDGE stores reach 313 GB/s, so the 9x window replication is
done on-chip, not by DMA):
  1. load x shard -> SBUF xt[p = h + 64*(b%2), free = (b//2)*4096 + w*64 + k]
     (4 HWDGE DMAs, 16KB descriptors)
  2. PE matmuls in transpose mode (exact 0/1 routing, 2 cycles/row fp32):
     psum[r, (w', k)] = sum_h Id[h, r+i] * xt[h, (c0+w')*64+k]
     for each (c-chunk u of 6, kernel-row i, batch b): 132 matmuls, N<=512
  3. DVE+ACT weave copies expand the j-overlap:
     stage[r, c*2304 + b*576 + i*192 + j*64 + k] = psum[r, (c+j)*64 + k]
     stage = [62 partitions, 2 ping-pong buffers x 6*2304 f32]
  4. SWDGE stores: per chunk one DMA [[142848, 62], [1, csz*2304]]
     -> 62 descriptors x 55KB, ~313 GB/s
"""

import numpy as np

import concourse.bass as bass
import concourse.mybir as mybir
from concourse.ap import AP
from concourse.bass_utils import run_bass_kernel_spmd

# Full-problem constants (hardcoded per harness contract)
B, H, W, C = 32, 64, 64, 64
K = 3
R = H - K + 1  # 62
NCORES = 8
BS = B // NCORES  # 4

WC = W * C                    # 4096
ROW = 2 * WC                  # 8192 f32 per partition of xt
RUN = BS * K * K * C          # 2304 f32 per (r, c) output run
OUT_STRIDE_R = R * RUN        # 142848
CHUNKS = [(c0, min(6, R - c0)) for c0 in range(0, R, 6)]  # 11 chunks
NMM = len(CHUNKS) * K * BS    # 132 matmuls
BUF = 6 * RUN                 # f32 per stage buffer (ping-pong)
PSROW = 4096                  # psum f32 per partition (8 banks x 512)


def _build_nc() -> bass.Bass:
    nc = bass.Bass(target_bir_lowering=False)
    x = nc.dram_tensor("x", [BS, H, W, C], mybir.dt.float32, kind="ExternalInput")
    out = nc.dram_tensor(
        "out", [R * R * BS, K, K, C], mybir.dt.float32, kind="ExternalOutput"
    )

    mms = [
        (u, i, b)
        for u in range(len(CHUNKS))
        for i in range(K)
        for b in range(BS)
    ]

    with (
        nc.sbuf_tensor("xt", [128, ROW], mybir.dt.float32) as xt,
        nc.sbuf_tensor("stage", [128, 2 * BUF], mybir.dt.float32) as stage,
        nc.sbuf_tensor("iop", [128, 64], mybir.dt.float32) as iop,
        nc.sbuf_tensor("iof", [128, 64], mybir.dt.float32) as iof,
        nc.sbuf_tensor("ident", [128, 64], mybir.dt.float32) as ident,
        nc.psum_tensor("ps", [128, PSROW], mybir.dt.float32) as ps,
        nc.semaphore("l0") as l0,
        nc.semaphore("isem") as isem,
        nc.semaphore("mm_sem") as mm_sem,
        nc.semaphore("cp0") as cp0,
        nc.semaphore("cp1") as cp1,
        nc.semaphore("st_e") as st_e,
        nc.semaphore("st_o") as st_o,
        nc.Block() as block,
    ):
        def copy_aps(n):
            u, i, b = mms[n]
            c0, csz = CHUNKS[u]
            src = AP(
                ps,
                (n % 8) * 512,
                [[PSROW, R], [C, csz], [C, K], [1, C]],
            )
            dst = AP(
                stage,
                (u % 2) * BUF + b * K * K * C + i * K * C,
                [[2 * BUF, R], [RUN, csz], [C, K], [1, C]],
            )
            return dst, src

        @block.sync
        def _(sync):
            for b in range(BS):
                src = AP(x, b * H * WC, [[WC, H], [1, WC]])
                dst = AP(xt, (H * (b % 2)) * ROW + (b // 2) * WC, [[ROW, H], [1, WC]])
                sync.dma_start(dst, src).then_inc(l0, 16)

        @block.gpsimd
        def _(gp):
            gp.iota(
                AP(iop, 0, [[64, 128], [1, 64]]),
                [[0, 64]],
                channel_multiplier=1,
                allow_small_or_imprecise_dtypes=True,
            ).then_inc(isem, 1)
            gp.iota(
                AP(iof, 0, [[64, 64], [1, 64]]),
                [[1, 64]],
                channel_multiplier=0,
                allow_small_or_imprecise_dtypes=True,
            ).then_inc(isem, 1)
            gp.iota(
                AP(iof, 64 * 64, [[64, 64], [1, 64]]),
                [[1, 64]],
                base=64,
                channel_multiplier=0,
                allow_small_or_imprecise_dtypes=True,
            ).then_inc(isem, 1)
            for u, (c0, csz) in enumerate(CHUNKS):
                gp.wait_ge(cp0, 6 * (u + 1))
                gp.wait_ge(cp1, 6 * (u + 1))
                src = AP(stage, (u % 2) * BUF, [[2 * BUF, R], [1, csz * RUN]])
                dst = AP(out, c0 * RUN, [[OUT_STRIDE_R, R], [1, csz * RUN]])
                gp.dma_start(dst, src).then_inc((st_e, st_o)[u % 2], 16)
            gp.wait_ge(st_e, 16 * 6)
            gp.wait_ge(st_o, 16 * 5)

        @block.vector
        def _(vec):
            vec.wait_ge(isem, 3)
            vec.tensor_tensor(
                AP(ident, 0, [[64, 128], [1, 64]]),
                AP(iop, 0, [[64, 128], [1, 64]]),
                AP(iof, 0, [[64, 128], [1, 64]]),
                mybir.AluOpType.is_equal,
            ).then_inc(isem, 1)
            nc_ = 0
            for n in range(NMM):
                if n % 2 != 0:
                    continue
                u = mms[n][0]
                vec.wait_ge(mm_sem, n + 1)
                if u >= 2:
                    vec.wait_ge((st_e, st_o)[u % 2], 16 * (u // 2))
                dst, src = copy_aps(n)
                vec.tensor_copy(dst, src).then_inc(cp0, 1)
                nc_ += 1

        @block.scalar
        def _(sc):
            for n in range(NMM):
                if n % 2 != 1:
                    continue
                u = mms[n][0]
                sc.wait_ge(mm_sem, n + 1)
                if u >= 2:
                    sc.wait_ge((st_e, st_o)[u % 2], 16 * (u // 2))
                dst, src = copy_aps(n)
                sc.copy(dst, src).then_inc(cp1, 1)

        @block.tensor
        def _(te):
            te.wait_ge(isem, 4)
            te.wait_ge(l0, 16 * BS)
            for n in range(NMM):
                u, i, b = mms[n]
                c0, csz = CHUNKS[u]
                if n >= 8:
                    j = n - 8
                    te.wait_ge((cp0, cp1)[j % 2], j // 2 + 1)
                nfree = (csz + 2) * C
                out_ap = AP(ps, (n % 8) * 512, [[PSROW, R], [1, nfree]])
                lhsT = AP(ident, (b % 2) * H * 64 + i, [[64, 64], [1, R]])
                rhs = AP(
                    xt,
                    (b % 2) * H * ROW + (b // 2) * WC + c0 * C,
                    [[ROW, H], [C, csz + 2], [1, C]],
                )
                te.matmul(out_ap, lhsT, rhs).then_inc(mm_sem, 1)

    return nc


_NC = None


def _get_nc():
    global _NC
    if _NC is None:
        _NC = _build_nc()
    return _NC


def kernel(x: np.ndarray, **_run_kwargs) -> np.ndarray:
    assert x.shape == (B, H, W, C), x.shape
    nc = _get_nc()
    x = np.ascontiguousarray(x, dtype=np.float32)
    in_maps = [{"x": x[d * BS : (d + 1) * BS]} for d in range(NCORES)]
    res = run_bass_kernel_spmd(nc, in_maps, list(range(NCORES)), **_run_kwargs)
    outs = [res.results[d]["out"].reshape(R * R, BS, K, K, C) for d in range(NCORES)]
    full = np.concatenate(outs, axis=1).reshape(R * R * B, K, K, C)
    if _run_kwargs:
        return full, res
    return full

